# revision 1
# baseline (speedup 1.0000x reference)
"""Trainium2 Bass kernel for CantorAttention.

Strategy
--------
The Cantor routes are a pure function of the (quantized) Cantor value of each
position: sorting positions by that value makes every query's 64-key route set
live inside a narrow (<=385-wide) window of the sorted order.  Sparse
attention therefore becomes dense *banded* attention after a host-side
permutation:

  host:   pi = argsort(cantor_val), permute x rows, transpose; build per
          128-query-tile 128-aligned windows of width 384 plus an additive
          bf16 mask (-30000 at non-selected slots).
  device: qkvT projection (fp32r matmuls), banded scores + mask (PE),
          exp+rowsum (ACT, fused accum), normalize (GPSIMD), PE-transpose of
          the probabilities into per-128-chunk column-major buffers, PV
          matmuls accumulating transposed attention output, and the output
          projection producing a partial (4-head) outT block.
  host:   sum the 4 partial outT blocks per batch, transpose, un-permute,
          add the output bias.

Sharding: batch x head-block -> 8 cores (core c: b = c//4, heads 4*(c%4)..).
"""

import sys

sys.path.insert(0, "/opt/trn_rl_repo")

import numpy as np

B, S, DIM = 2, 2048, 1024
HEADS, DH = 16, 64
K_NEI = 64
N_CORES = 8
HPC = 4            # heads per core
QT = 128           # query tile (rows per tile)
NT = S // QT       # 16 query tiles
SUP = 4            # query tiles per supertile (PV batch of 512 queries)
NSUP = NT // SUP

_CACHE = {}


def _cantor_val(seq_len, depth=8):
    pos = np.arange(seq_len, dtype=np.float64)
    x = pos / max(1, seq_len - 1)
    x = np.clip(x, 1e-6, 1.0 - 1e-6)
    val = np.zeros_like(x)
    factor = 0.5
    for _ in range(depth):
        xs = x * 3.0
        digit = np.floor(xs)
        x = xs - digit
        val = val + (digit == 2.0).astype(np.float64) * factor
        factor *= 0.5
    return np.clip(val, 0.0, 1.0)


def _geometry(routes):
    """Window geometry from the runtime routes array."""
    val = _cantor_val(S)
    pi = np.argsort(val, kind="stable").astype(np.int64)
    rank = np.empty(S, np.int64)
    rank[pi] = np.arange(S)
    kr = rank[np.asarray(routes, np.int64)][pi]      # [S, K] key ranks, query-rank order
    lo = kr.min(1)
    hi = kr.max(1) + 1
    for win in (384, 512):
        a = np.zeros(NT, np.int64)
        ok = True
        for t in range(NT):
            l = int(lo[t * QT:(t + 1) * QT].min())
            h = int(hi[t * QT:(t + 1) * QT].max())
            a[t] = min(l // 128, (S - win) // 128)
            if h > a[t] * 128 + win:
                ok = False
                break
        if ok:
            return pi, rank, kr, a, win
    raise ValueError("routes structure incompatible with banded-window kernel")


def _build_module(a, win, loop_n=1, phases="ACD", cheat_dma=False):
    from contextlib import nullcontext

    from concourse import bacc, tile, mybir
    from concourse.masks import make_identity

    f32 = mybir.dt.float32
    f32r = mybir.dt.float32r
    bf16 = mybir.dt.bfloat16
    AF = mybir.ActivationFunctionType
    NCH = win // 128                      # chunks per window
    a = [int(v) for v in a]

    # chunk -> [first tile, last tile] using it
    chunk_tiles = {}
    for t in range(NT):
        for j in range(NCH):
            c = a[t] + j
            lo_t, hi_t = chunk_tiles.get(c, (t, t))
            chunk_tiles[c] = (min(lo_t, t), max(hi_t, t))

    nc = bacc.Bacc("TRN2", target_bir_lowering=False, debug=False)
    xT = nc.dram_tensor("xT", [DIM, S], f32r, kind="ExternalInput").ap()
    wq = nc.dram_tensor("wq", [DIM, 3 * HPC * DH], f32r, kind="ExternalInput").ap()
    bq = nc.dram_tensor("bq", [3 * HPC * DH, 1], f32, kind="ExternalInput").ap()
    wo = nc.dram_tensor("wo", [HPC * DH, DIM], f32r, kind="ExternalInput").ap()
    mask = nc.dram_tensor("mask", [QT, NT * win], bf16, kind="ExternalInput").ap()
    outp = nc.dram_tensor("outp", [DIM, S], f32, kind="ExternalOutput").ap()

    NQKV = 3 * HPC * DH                  # 768 rows of qkvT
    NMT = NQKV // 128                    # 6 row-tiles of qkvT

    with tile.TileContext(nc) as tc:
        with tc.tile_pool(name="persist", bufs=1) as pp:
            id32 = pp.tile([128, 128], f32)
            make_identity(nc, id32)
            id_r = pp.tile([128, 128], f32r)
            nc.vector.tensor_copy(id_r, id32)
            id_b = pp.tile([128, 128], bf16)
            nc.vector.tensor_copy(id_b, id32)
            mask_sb = pp.tile([QT, NT * win], bf16)
            nc.sync.dma_start(out=mask_sb, in_=mask)
            bq_sb = []
            for m in range(NMT):
                bt = pp.tile([128, 1], f32, tag=f"bq{m}", name=f"bq{m}")
                nc.sync.dma_start(out=bt, in_=bq[m * 128:(m + 1) * 128, :])
                bq_sb.append(bt)
            qkvT = [pp.tile([128, S], f32r, tag=f"qkvT{m}", name=f"qkvT{m}")
                    for m in range(NMT)]
            attn_outT = [pp.tile([128, S], f32r, tag=f"aout{p}", name=f"aout{p}")
                         for p in range(2)]
            wo_sb = []
            for p2 in range(2):
                wt = pp.tile([128, DIM], f32r, tag=f"wo{p2}", name=f"wo{p2}")
                nc.sync.dma_start(out=wt, in_=wo[p2 * 128:(p2 + 1) * 128, :])
                wo_sb.append(wt)

            loop_cm = tc.For_i(0, loop_n, 1) if loop_n > 1 else nullcontext()
            with loop_cm:
                # ------------- Phase A: qkvT = wq.T @ xT (+bias) -------------
                if "A" in phases:
                    with tc.tile_pool(name="phA", bufs=1) as pa, \
                         tc.tile_pool(name="phAx", bufs=2) as pax, \
                         tc.tile_pool(name="psA", bufs=3, space="PSUM") as psa:
                        wq_sb = []
                        for kk in range(8):
                            wt = pa.tile([128, NQKV], f32r, tag=f"wq{kk}",
                                         name=f"wq{kk}")
                            nc.sync.dma_start(out=wt, in_=wq[kk * 128:(kk + 1) * 128, :])
                            wq_sb.append(wt)
                        xt_prev = None
                        for n in range(4):
                            if cheat_dma and n > 0:
                                xt = xt_prev
                            else:
                                xt = []
                                for kk in range(8):
                                    t_ = pax.tile([128, 512], f32r, tag=f"x{kk}",
                                                  name=f"x{kk}_{n}")
                                    nc.sync.dma_start(
                                        out=t_,
                                        in_=xT[kk * 128:(kk + 1) * 128,
                                               n * 512:(n + 1) * 512])
                                    xt.append(t_)
                                xt_prev = xt
                            for m in (4, 5, 2, 3, 0, 1):
                                ps = psa.tile([128, 512], f32, tag="ps")
                                for kk in range(8):
                                    nc.tensor.matmul(
                                        ps, wq_sb[kk][:, m * 128:(m + 1) * 128], xt[kk],
                                        start=(kk == 0), stop=(kk == 7))
                                if (n + m) % 2 == 0:
                                    nc.scalar.activation(
                                        out=qkvT[m][:, n * 512:(n + 1) * 512],
                                        in_=ps, func=AF.Identity, bias=bq_sb[m])
                                else:
                                    nc.vector.tensor_scalar_add(
                                        qkvT[m][:, n * 512:(n + 1) * 512], ps,
                                        bq_sb[m])

                # ---------- Phases B+C: V transpose + banded attention ----------
                if "C" in phases:
                    with tc.tile_pool(name="phC", bufs=1) as pc, \
                         tc.tile_pool(name="pexp_pool", bufs=12) as pe_pool, \
                         tc.tile_pool(name="pt_pool", bufs=18) as pt_pool, \
                         tc.tile_pool(name="small", bufs=16) as sm_pool, \
                         tc.tile_pool(name="psB", bufs=3, space="PSUM") as psb, \
                         tc.tile_pool(name="psS", bufs=3, space="PSUM") as pss, \
                         tc.tile_pool(name="psO", bufs=2, space="PSUM") as pso:
                        V_sb = [pc.tile([128, 2 * 128], f32r, tag=f"V{cc}",
                                        name=f"V{cc}") for cc in range(NT)]
                        for cc in range(NT):
                            pv = psb.tile([128, 512], f32r, tag="ptr",
                                          name=f"pv{cc}")
                            for s_ in range(2):
                                nc.tensor.transpose(
                                    pv[:, s_ * 128:(s_ + 1) * 128],
                                    qkvT[4 + s_][:, cc * 128:(cc + 1) * 128], id_r)
                            if cc % 2 == 0:
                                nc.vector.tensor_copy(V_sb[cc], pv[:, 0:256])
                            else:
                                nc.scalar.copy(V_sb[cc], pv[:, 0:256])

                        aoutB = [pc.tile([64, S], f32r, tag=f"aoutB{i}",
                                         name=f"aoutB{i}") for i in range(2)]

                        def stage1(h, u, pn):
                            poff = (h % 2) * 64
                            qTh = qkvT[h // 2]
                            kTh = qkvT[2 + h // 2]
                            den_u = sm_pool.tile([128, SUP], f32, tag="den",
                                                 name=f"den{h}_{u}")
                            rec_u = sm_pool.tile([128, SUP], f32, tag="rec",
                                                 name=f"rec{h}_{u}")
                            pexps = {}
                            for t in range(u * SUP, (u + 1) * SUP):
                                w0 = a[t] * 128
                                ps_s = pss.tile([128, win], f32, tag="sc",
                                                name=f"sc{h}_{t}")
                                nc.tensor.matmul(
                                    ps_s,
                                    qTh[poff:poff + 64, t * 128:(t + 1) * 128],
                                    kTh[poff:poff + 64, w0:w0 + win],
                                    start=True, stop=False, skip_group_check=True)
                                nc.tensor.matmul(
                                    ps_s, id_b, mask_sb[:, t * win:(t + 1) * win],
                                    start=False, stop=True, skip_group_check=True)
                                pexp = pe_pool.tile([128, win], f32, tag="pexp",
                                                    name=f"pexp{h}_{t}")
                                i = t - u * SUP
                                nc.scalar.activation(out=pexp, in_=ps_s,
                                                     func=AF.Exp,
                                                     accum_out=den_u[:, i:i + 1])
                                pexps[t] = pexp
                            nc.vector.reciprocal(rec_u, den_u)
                            for t in range(u * SUP, (u + 1) * SUP):
                                i = t - u * SUP
                                pnorm = pe_pool.tile([128, win], f32r, tag="pnorm",
                                                     name=f"pnorm{h}_{t}")
                                nc.vector.tensor_scalar_mul(pnorm, pexps[t],
                                                            rec_u[:, i:i + 1])
                                pn[t] = pnorm

                        def stage2(h, u, pn, pt_tiles):
                            poff = (h % 2) * 64
                            # chunk-major transposes into a per-(chunk,unit) PSUM
                            # tile, then ONE copy per chunk into its SBUF buffer
                            tiles_u = range(u * SUP, (u + 1) * SUP)
                            cset = sorted({a[t] + j for t in tiles_u
                                           for j in range(NCH)})
                            for c in cset:
                                t0c, t1c = chunk_tiles[c]
                                if c not in pt_tiles:
                                    pt_tiles[c] = pt_pool.tile(
                                        [128, (t1c - t0c + 1) * 128], f32r,
                                        tag="pt", name=f"pt_h{h}_c{c}")
                                tlo = max(t0c, u * SUP)
                                thi = min(t1c, (u + 1) * SUP - 1)
                                wdt = (thi - tlo + 1) * 128
                                ptp = psb.tile([128, 512], f32r, tag="ptr",
                                               name=f"ptr{h}_{u}_{c}")
                                for t in range(tlo, thi + 1):
                                    nc.tensor.transpose(
                                        ptp[:, (t - tlo) * 128:(t - tlo + 1) * 128],
                                        pn[t][:, (c - a[t]) * 128:
                                              (c - a[t] + 1) * 128], id_r)
                                nc.vector.tensor_copy(
                                    pt_tiles[c][:, (tlo - t0c) * 128:
                                                (thi - t0c + 1) * 128],
                                    ptp[:, 0:wdt])
                            # PV pieces: widest chunk start=True, straddlers split
                            chunks_u = sorted({a[t] + j
                                               for t in range(u * SUP, (u + 1) * SUP)
                                               for j in range(NCH)})
                            ranges = []
                            for c in chunks_u:
                                t0c, t1c = chunk_tiles[c]
                                tlo = max(t0c, u * SUP)
                                thi = min(t1c, (u + 1) * SUP - 1)
                                ranges.append((c, tlo * 128 - u * 512,
                                               (thi + 1) * 128 - u * 512))
                            first = max(ranges, key=lambda r: r[2] - r[1])
                            pieces = [first]
                            wlo, whi = first[1], first[2]
                            for c, o0, o1 in sorted(
                                    (r for r in ranges if r is not first),
                                    key=lambda r: r[1]):
                                for p0, p1 in ((o0, min(o1, wlo)),
                                               (max(o0, wlo), min(o1, whi)),
                                               (max(o0, whi), o1)):
                                    if p1 > p0:
                                        pieces.append((c, p0, p1))
                                wlo, whi = min(wlo, o0), max(whi, o1)
                            po = pso.tile([128, 512], f32, tag="po",
                                          name=f"po{h}_{u}")
                            for i_p, (c, o0, o1) in enumerate(pieces):
                                t0c, _ = chunk_tiles[c]
                                r0 = o0 + u * 512 - t0c * 128
                                r1 = o1 + u * 512 - t0c * 128
                                nc.tensor.matmul(
                                    po[0:64, o0:o1],
                                    V_sb[c][:, h * 64:(h + 1) * 64],
                                    pt_tiles[c][:, r0:r1],
                                    start=(i_p == 0),
                                    stop=(i_p == len(pieces) - 1),
                                    skip_group_check=True)
                            if poff == 0:
                                dst = attn_outT[h // 2][0:64, u * 512:(u + 1) * 512]
                            else:
                                dst = aoutB[h // 2][:, u * 512:(u + 1) * 512]
                            nc.vector.tensor_copy(dst, po[0:64, :])
                            if poff != 0:
                                nc.sync.dma_start(
                                    out=attn_outT[h // 2][64:128,
                                                          u * 512:(u + 1) * 512],
                                    in_=aoutB[h // 2][:, u * 512:(u + 1) * 512])

                        units = [(h, u) for h in range(HPC) for u in range(NSUP)]
                        DELAY = 2
                        pn_store = {}
                        pt_store = {h: {} for h in range(HPC)}
                        pending = []
                        for h, u in units:
                            pn = {}
                            stage1(h, u, pn)
                            pn_store[(h, u)] = pn
                            pending.append((h, u))
                            if len(pending) > DELAY:
                                ph, pu = pending.pop(0)
                                stage2(ph, pu, pn_store.pop((ph, pu)), pt_store[ph])
                        for ph, pu in pending:
                            stage2(ph, pu, pn_store.pop((ph, pu)), pt_store[ph])

                # ------------- Phase D: outp = wo.T @ attn_outT -------------
                if "D" in phases:
                    with tc.tile_pool(name="phD", bufs=2) as pd, \
                         tc.tile_pool(name="psD", bufs=2, space="PSUM") as psd:
                        for mm in range(8):
                            st = pd.tile([128, S], f32, tag="st")
                            for n in range(4):
                                ps = psd.tile([128, 512], f32, tag="pod")
                                for p2 in range(2):
                                    nc.tensor.matmul(
                                        ps, wo_sb[p2][:, mm * 128:(mm + 1) * 128],
                                        attn_outT[p2][:, n * 512:(n + 1) * 512],
                                        start=(p2 == 0), stop=(p2 == 1))
                                if (mm + n) % 2 == 0:
                                    nc.scalar.copy(st[:, n * 512:(n + 1) * 512], ps)
                                else:
                                    nc.vector.tensor_copy(st[:, n * 512:(n + 1) * 512],
                                                          ps)
                            nc.sync.dma_start(out=outp[mm * 128:(mm + 1) * 128, :],
                                              in_=st)

    nc.compile()
    return nc


def _get_module(a, win):
    key = (tuple(int(v) for v in a), int(win))
    if key not in _CACHE:
        _CACHE[key] = _build_module(a, win)
    return _CACHE[key]


def kernel(x, routes, qkv_w, qkv_b, out_w, out_b):
    import ml_dtypes
    from concourse.bass_utils import run_bass_kernel_spmd

    x = np.ascontiguousarray(np.asarray(x, np.float32))
    routes = np.asarray(routes)
    qkv_w = np.asarray(qkv_w, np.float32)
    qkv_b = np.asarray(qkv_b, np.float32)
    out_w = np.asarray(out_w, np.float32)
    out_b = np.asarray(out_b, np.float32)

    pi, rank, kr, a, win = _geometry(routes)
    SCALE = 1.0 / float(np.sqrt(DH))

    # masks [QT, NT*win] additive bf16, shared by all cores
    mask_np = np.full((NT, QT, win), -30000.0, np.float32)
    rows = np.repeat(np.arange(QT), K_NEI)
    for t in range(NT):
        krt = (kr[t * QT:(t + 1) * QT] - a[t] * 128).ravel()
        mask_np[t, rows, krt] = 0.0
    mask_np = np.ascontiguousarray(
        mask_np.transpose(1, 0, 2).reshape(QT, NT * win)).astype(ml_dtypes.bfloat16)

    xT_b = [np.ascontiguousarray(x[b][pi].T) for b in range(B)]

    in_maps = []
    for c in range(N_CORES):
        b = c // (N_CORES // B)
        hb = c % (N_CORES // B)
        heads = range(hb * HPC, (hb + 1) * HPC)
        w_rows = []
        b_rows = []
        for sect, scale in ((0, SCALE), (1, 1.0), (2, 1.0)):
            for h in heads:
                r0 = sect * DIM + h * DH
                w_rows.append(qkv_w[r0:r0 + DH] * scale)
                b_rows.append(qkv_b[r0:r0 + DH] * scale)
        wq_c = np.ascontiguousarray(np.concatenate(w_rows, 0).T)          # [DIM, 768]
        bq_c = np.concatenate(b_rows, 0).reshape(-1, 1).astype(np.float32)
        wo_c = np.ascontiguousarray(out_w[:, hb * HPC * DH:(hb + 1) * HPC * DH].T)
        in_maps.append({
            "xT": xT_b[b],
            "wq": wq_c,
            "bq": bq_c,
            "wo": wo_c,
            "mask": mask_np,
        })

    nc = _get_module(a, win)
    res = run_bass_kernel_spmd(nc, in_maps, core_ids=list(range(N_CORES)))

    out = np.empty((B, S, DIM), np.float32)
    for b in range(B):
        cores = [c for c in range(N_CORES) if c // (N_CORES // B) == b]
        outT = res.results[cores[0]]["outp"].astype(np.float32)
        for c in cores[1:]:
            outT = outT + res.results[c]["outp"]
        rows_sorted = outT.T                      # [S, DIM] in rank order
        tmp = np.empty_like(rows_sorted)
        tmp[pi] = rows_sorted
        out[b] = tmp + out_b[None, :]
    return out



# revision 23
# speedup vs baseline: 1.6987x; 1.6987x over previous
"""Trainium2 Bass kernel for CantorAttention.

Strategy
--------
The Cantor routes are a pure function of the (quantized) Cantor value of each
position: sorting positions by that value makes every query's 64-key route set
live inside a narrow (<=385-wide) window of the sorted order.  Sparse
attention therefore becomes dense *banded* attention after a host-side
permutation.

This version computes the banded scores directly in transposed [key, query]
layout, one 128-key chunk at a time, which removes every probability
transpose from the hot path:

  host:   pi = argsort(cantor_val), permute x rows, transpose, cast bf16;
          build a per-query-tile 0/1 transposed key mask.
  device: Phase A: qT/kT projections ([d, s], bf16) and V in [s, d] layout
          with a constant-ones column per head (bias of V folded into the
          host-side output bias).
          Phase C: per (head, query-tile): S^T chunks = kT_chunk^T @ qT_tile
          on PE, exp on ACT, 0/1 mask multiply on DVE (bf16 2x/4x mode), then
          PV with the exp tile as the stationary operand: out[q, 64+1] where
          the +1 column (ones in V) accumulates the softmax denominator.
          Normalisation is then a per-partition reciprocal+scale.
          Per supertile: PE-transpose of the normalised attention output into
          [d, s] layout, then the output projection (Phase D).
  host:   sum the 4 partial outT blocks per batch, transpose, un-permute,
          add the (output + folded V) bias.

Sharding: batch x head-block -> 8 cores (core c: b = c//4, heads 4*(c%4)..).
"""

import sys

sys.path.insert(0, "/opt/trn_rl_repo")

import numpy as np

B, S, DIM = 2, 2048, 1024
HEADS, DH = 16, 64
K_NEI = 64
N_CORES = 8
HPC = 4            # heads per core
QT = 128           # query tile (rows per tile)
NT = S // QT       # 16 query tiles
SUP = 4            # query tiles per supertile
NSUP = NT // SUP
MAXCH = 3          # chunk slots per tile in the mask layout

_CACHE = {}


def _cantor_val(seq_len, depth=8):
    pos = np.arange(seq_len, dtype=np.float64)
    x = pos / max(1, seq_len - 1)
    x = np.clip(x, 1e-6, 1.0 - 1e-6)
    val = np.zeros_like(x)
    factor = 0.5
    for _ in range(depth):
        xs = x * 3.0
        digit = np.floor(xs)
        x = xs - digit
        val = val + (digit == 2.0).astype(np.float64) * factor
        factor *= 0.5
    return np.clip(val, 0.0, 1.0)


def _geometry(routes):
    """Window geometry from the runtime routes array.

    Returns (pi, rank, kr, a, nch): permutation, ranks, routed key ranks in
    query-rank order, per-tile 128-aligned window start (in chunks) and
    per-tile chunk count (<= MAXCH).
    """
    val = _cantor_val(S)
    pi = np.argsort(val, kind="stable").astype(np.int64)
    rank = np.empty(S, np.int64)
    rank[pi] = np.arange(S)
    kr = rank[np.asarray(routes, np.int64)][pi]      # [S, K] key ranks
    a = np.zeros(NT, np.int64)
    nch = np.zeros(NT, np.int64)
    for t in range(NT):
        l = int(kr[t * QT:(t + 1) * QT].min())
        h = int(kr[t * QT:(t + 1) * QT].max()) + 1
        a[t] = l // 128
        nch[t] = -(-(h - a[t] * 128) // 128)
        if nch[t] > MAXCH:
            raise ValueError("routes structure incompatible with banded kernel")
    return pi, rank, kr, a, nch


def _build_module(a, nch, loop_n=1):
    from contextlib import nullcontext

    from concourse import bacc, tile, mybir
    from concourse.masks import make_identity

    f32 = mybir.dt.float32
    bf16 = mybir.dt.bfloat16
    AF = mybir.ActivationFunctionType
    a = [int(v) for v in a]
    nch = [int(v) for v in nch]

    nc = bacc.Bacc("TRN2", target_bir_lowering=False, debug=False)
    xT = nc.dram_tensor("xT", [DIM, S], bf16, kind="ExternalInput").ap()
    wall = nc.dram_tensor("wall", [DIM, 3 * HPC * DH], bf16,
                          kind="ExternalInput").ap()
    bqk = nc.dram_tensor("bqk", [128, 4], f32, kind="ExternalInput").ap()
    wo = nc.dram_tensor("wo", [HPC * DH, DIM], bf16, kind="ExternalInput").ap()
    maskT = nc.dram_tensor("maskT", [QT, NT * MAXCH * QT], bf16,
                           kind="ExternalInput").ap()
    outp = nc.dram_tensor("outp", [DIM, S], bf16, kind="ExternalOutput").ap()

    NQK = 2 * HPC * DH                   # 512 rows of q|k in qkT
    NMT = NQK // 128                     # 4 row-tiles of qkT

    with tile.TileContext(nc) as tc:
        with tc.tile_pool(name="persist", bufs=1) as pp:
            id32 = pp.tile([128, 128], f32)
            make_identity(nc, id32)
            id_b = pp.tile([128, 128], bf16)
            nc.vector.tensor_copy(id_b, id32)
            maskT_sb = pp.tile([QT, NT * MAXCH * QT], bf16)
            bqk_sb = pp.tile([128, NMT], f32, tag="bqk", name="bqk")
            wo_sb = []
            for p2 in range(2):
                wt = pp.tile([128, DIM], bf16, tag=f"wo{p2}", name=f"wo{p2}")
                wo_sb.append(wt)
            # qT/kT: row-tile m holds heads 2m, 2m+1 (m 0-1: q, 2-3: k)
            qkT = [pp.tile([128, S], bf16, tag=f"qkT{m}", name=f"qkT{m}")
                   for m in range(NMT)]
            # V chunks in [s, d] layout: 4 heads x (64 V cols + ones col)
            V_sb = [pp.tile([128, HPC * (DH + 1)], bf16, tag=f"V{cc}",
                            name=f"V{cc}") for cc in range(NT)]
            for cc in range(NT):
                nc.gpsimd.memset(V_sb[cc], 1.0)
            attn_sb = pp.tile([QT, NT * HPC * DH], bf16, tag="attn",
                              name="attn")
            attn_outT = [pp.tile([128, S], bf16, tag=f"aout{p}",
                                 name=f"aout{p}") for p in range(2)]

            loop_cm = tc.For_i(0, loop_n, 1) if loop_n > 1 else nullcontext()
            with loop_cm:
                # one software-pipelined scope: A(u+1) overlaps C(u)/D(u)
                with tc.tile_pool(name="phA", bufs=1) as pa, \
                     tc.tile_pool(name="pexp", bufs=10) as pe_pool, \
                     tc.tile_pool(name="prec", bufs=4) as prec_pool, \
                     tc.tile_pool(name="pstg", bufs=4) as pstg_pool, \
                     tc.tile_pool(name="psA", bufs=2, space="PSUM") as psa, \
                     tc.tile_pool(name="psS", bufs=2, space="PSUM") as pss, \
                     tc.tile_pool(name="psO", bufs=2, space="PSUM") as pso, \
                     tc.tile_pool(name="psD", bufs=2, space="PSUM") as psd:
                    wall_sb = []     # [wqk | wv] fused weight tiles
                    xt_sb = []
                    for kk in range(8):
                        wt = pa.tile([128, NQK + HPC * DH], bf16,
                                     tag=f"wall{kk}", name=f"wall{kk}")
                        nc.sync.dma_start(out=wt,
                                          in_=wall[kk * 128:(kk + 1) * 128, :])
                        wall_sb.append(wt)
                        t_ = pa.tile([128, S], bf16, tag=f"x{kk}",
                                     name=f"x{kk}")
                        nc.sync.dma_start(out=t_[:, 0:1024],
                                          in_=xT[kk * 128:(kk + 1) * 128,
                                                 0:1024])
                        xt_sb.append(t_)
                    nc.sync.dma_start(out=bqk_sb, in_=bqk)
                    for kk in range(8):
                        nc.sync.dma_start(out=xt_sb[kk][:, 1024:S],
                                          in_=xT[kk * 128:(kk + 1) * 128,
                                                 1024:S])
                    for p2 in range(2):
                        nc.sync.dma_start(out=wo_sb[p2],
                                          in_=wo[p2 * 128:(p2 + 1) * 128, :])
                    nc.sync.dma_start(out=maskT_sb, in_=maskT)

                    def phase_a(n):
                        for m in range(NMT):
                            ps = psa.tile([128, 512], f32, tag="ps",
                                          name=f"psa{n}_{m}")
                            for kk in range(8):
                                nc.tensor.matmul(
                                    ps, wall_sb[kk][:, m * 128:(m + 1) * 128],
                                    xt_sb[kk][:, n * 512:(n + 1) * 512],
                                    start=(kk == 0), stop=(kk == 7))
                            nc.vector.tensor_scalar_add(
                                qkT[m][:, n * 512:(n + 1) * 512], ps,
                                bqk_sb[m])
                        for j in range(4):
                            cc = n * 4 + j
                            pvt = psa.tile([128, 512], f32, tag="ps",
                                           name=f"psv{cc}")
                            pv = pvt[:, 0:HPC * DH]
                            for kk in range(8):
                                nc.tensor.matmul(
                                    pv, xt_sb[kk][:, cc * 128:(cc + 1) * 128],
                                    wall_sb[kk][:, NQK:], start=(kk == 0),
                                    stop=(kk == 7))
                            # scatter 4x64 head blocks into 4x65 slots
                            nc.gpsimd.tensor_copy(
                                V_sb[cc].rearrange("p (h e) -> p h e",
                                                   h=HPC)[:, :, 0:DH],
                                pv.rearrange("p (h e) -> p h e", h=HPC))

                    def a_qk(n, m):
                        ps = psa.tile([128, 512], f32, tag="ps",
                                      name=f"psa{n}_{m}")
                        for kk in range(8):
                            nc.tensor.matmul(
                                ps, wall_sb[kk][:, m * 128:(m + 1) * 128],
                                xt_sb[kk][:, n * 512:(n + 1) * 512],
                                start=(kk == 0), stop=(kk == 7))
                        nc.vector.tensor_scalar_add(
                            qkT[m][:, n * 512:(n + 1) * 512], ps,
                            bqk_sb[:, m:m + 1])

                    def a_v(cc):
                        pvt = psa.tile([128, 512], f32, tag="ps",
                                       name=f"psv{cc}")
                        pv = pvt[:, 0:HPC * DH]
                        for kk in range(8):
                            nc.tensor.matmul(
                                pv, xt_sb[kk][:, cc * 128:(cc + 1) * 128],
                                wall_sb[kk][:, NQK:], start=(kk == 0),
                                stop=(kk == 7))
                        # scatter 4x64 head blocks into 4x65 slots
                        nc.vector.tensor_copy(
                            V_sb[cc].rearrange("p (h e) -> p h e",
                                               h=HPC)[:, :, 0:DH],
                            pv.rearrange("p (h e) -> p h e", h=HPC))

                    def stage1(u, h, filler):
                        """S^T chunks + exp + mask for unit (u, h)."""
                        poff = (h % 2) * 64
                        qTh = qkT[h // 2]
                        kTh = qkT[2 + h // 2]
                        pexp = pe_pool.tile([128, SUP * MAXCH * 128], bf16,
                                            tag="pexp", name=f"pexp{u}_{h}")
                        for i, t in enumerate(range(u * SUP, (u + 1) * SUP)):
                            if i >= 2:
                                filler(i - 2)
                            st = pss.tile([128, MAXCH * 128], f32, tag="st",
                                          name=f"st{h}_{t}")
                            for j in range(nch[t]):
                                c = a[t] + j
                                nc.tensor.matmul(
                                    st[:, j * 128:(j + 1) * 128],
                                    kTh[poff:poff + 64, c * 128:(c + 1) * 128],
                                    qTh[poff:poff + 64, t * 128:(t + 1) * 128],
                                    start=(j == 0), stop=(j == nch[t] - 1),
                                    skip_group_check=True)
                            w = nch[t] * 128
                            nc.scalar.activation(
                                out=pexp[:, i * MAXCH * 128:
                                         i * MAXCH * 128 + w],
                                in_=st[:, 0:w], func=AF.Exp)
                        # one masking multiply for the whole supertile row
                        nc.vector.tensor_mul(
                            pexp, pexp,
                            maskT_sb[:, u * SUP * MAXCH * 128:
                                     (u + 1) * SUP * MAXCH * 128])
                        return pexp

                    def stage2(u, h, pexp):
                        """PV + denominators + normalisation for unit (u, h)."""
                        po = pso.tile([128, SUP * (DH + 1)], f32, tag="po",
                                      name=f"po{h}_{u}")
                        first = True
                        for i, t in enumerate(range(u * SUP, (u + 1) * SUP)):
                            for j in range(nch[t]):
                                c = a[t] + j
                                last = (i == SUP - 1 and j == nch[t] - 1)
                                nc.tensor.matmul(
                                    po[:, i * (DH + 1):(i + 1) * (DH + 1)],
                                    pexp[:, (i * MAXCH + j) * 128:
                                         (i * MAXCH + j + 1) * 128],
                                    V_sb[c][:, h * (DH + 1):(h + 1) * (DH + 1)],
                                    start=first, stop=last,
                                    skip_group_check=True)
                                first = False
                        po3 = po.rearrange("p (i e) -> p i e", i=SUP)
                        rec = prec_pool.tile([128, SUP], f32, tag="rec",
                                             name=f"rec{h}_{u}")
                        nc.vector.reciprocal(rec, po3[:, :, DH:DH + 1])
                        att3 = attn_sb.rearrange("p (t e) -> p t e",
                                                 e=HPC * DH)
                        nc.vector.scalar_tensor_tensor(
                            att3[:, u * SUP:(u + 1) * SUP,
                                 h * DH:(h + 1) * DH],
                            po3[:, :, 0:DH], 1.0,
                            rec[:, :, None].to_broadcast((128, SUP, DH)),
                            op0=mybir.AluOpType.mult,
                            op1=mybir.AluOpType.mult)

                    def tr(u):
                        """Transpose attn [q, d] -> [d, q], 2 tiles per pass."""
                        for pair in range(2):
                            t0 = u * SUP + pair * 2
                            ptr = psd.tile([128, 512], bf16, tag="pod",
                                           name=f"ptr{t0}")
                            for z in range(2):
                                for p2 in range(2):
                                    nc.tensor.transpose(
                                        ptr[:, (z * 2 + p2) * 128:
                                            (z * 2 + p2 + 1) * 128],
                                        attn_sb[:, (t0 + z) * 256 + p2 * 128:
                                                (t0 + z) * 256 +
                                                (p2 + 1) * 128],
                                        id_b)
                            ptr3 = ptr.rearrange("p (z two q) -> p z two q",
                                                 z=2, two=2)
                            for p2 in range(2):
                                nc.scalar.copy(
                                    attn_outT[p2][:, t0 * 128:(t0 + 2) * 128],
                                    ptr3[:, :, p2, :])

                    def d_mm(u, mm):
                        ps = psd.tile([128, 512], f32, tag="pod",
                                      name=f"psd{mm}_{u}")
                        for p2 in range(2):
                            nc.tensor.matmul(
                                ps, wo_sb[p2][:, mm * 128:(mm + 1) * 128],
                                attn_outT[p2][:, u * 512:(u + 1) * 512],
                                start=(p2 == 0), stop=(p2 == 1))
                        stg = pstg_pool.tile([128, 512], bf16, tag="stg",
                                             name=f"stg{mm}_{u}")
                        if mm % 2 == 0:
                            nc.scalar.copy(stg, ps)
                        else:
                            nc.vector.tensor_copy(stg, ps)
                        nc.sync.dma_start(
                            out=outp[mm * 128:(mm + 1) * 128,
                                     u * 512:(u + 1) * 512],
                            in_=stg)

                    # ---- software-pipelined emission ----
                    # prologue: projections for the first two column blocks
                    for n in (0, 1):
                        for m in range(NMT):
                            a_qk(n, m)
                        for j in range(4):
                            a_v(n * 4 + j)
                    # filler queue: remaining A work, then D picked up as
                    # transposes complete.  2 pieces drained per C-unit.
                    fillers = [lambda n=n, m=m: a_qk(n, m)
                               for n in (2, 3) for m in range(NMT)]
                    fillers[4:4] = [lambda cc=cc: a_v(cc) for cc in range(8, 12)]
                    fillers.extend(lambda cc=cc: a_v(cc) for cc in range(12, 16))
                    DELAY = 1
                    budget = [0]

                    def drain_one(slot=0):
                        if fillers and budget[0] > 0:
                            budget[0] -= 1
                            fillers.pop(0)()

                    pending = []
                    for u in range(NSUP):
                        for h in range(HPC):
                            unit = u * HPC + h
                            budget[0] = 2 if unit < 4 else 3
                            pexp = stage1(u, h, drain_one)
                            pending.append((u, h, pexp))
                            if len(pending) > DELAY:
                                pu, ph, pe_ = pending.pop(0)
                                stage2(pu, ph, pe_)
                                if ph == HPC - 1:
                                    tr(pu)
                                    fillers.extend(
                                        lambda uu=pu, mm=mm: d_mm(uu, mm)
                                        for mm in range(8))
                            drain_one()
                    for pu, ph, pe_ in pending:
                        stage2(pu, ph, pe_)
                        if ph == HPC - 1:
                            tr(pu)
                            fillers.extend(lambda uu=pu, mm=mm: d_mm(uu, mm)
                                           for mm in range(8))
                    for f in fillers:
                        f()

    nc.compile()
    return nc


def _get_module(a, nch):
    key = (tuple(int(v) for v in a), tuple(int(v) for v in nch))
    if key not in _CACHE:
        _CACHE[key] = _build_module(a, nch)
    return _CACHE[key]


def _prep_inputs(x, routes, qkv_w, qkv_b, out_w):
    """Returns (in_maps, pi) for the 8 cores."""
    import ml_dtypes

    bf = ml_dtypes.bfloat16
    pi, rank, kr, a, nch = _geometry(routes)
    SCALE = 1.0 / float(np.sqrt(DH))

    # transposed 0/1 key mask: [kk partition, (t, j, qq) free]
    mask_np = np.zeros((NT, MAXCH, 128, QT), np.float32)
    t_all = np.arange(S) // QT
    col = kr - (a[t_all] * 128)[:, None]             # [S, K] window col
    rows = np.repeat(t_all, K_NEI)
    qq = np.repeat(np.arange(S) % QT, K_NEI)
    colr = col.ravel()
    mask_np[rows, colr // 128, colr % 128, qq] = 1.0
    maskT_np = np.ascontiguousarray(
        mask_np.transpose(2, 0, 1, 3).reshape(128, NT * MAXCH * 128)).astype(bf)

    xT_b = [np.ascontiguousarray(x[b][pi].T).astype(bf) for b in range(B)]

    in_maps = []
    for c in range(N_CORES):
        b = c // (N_CORES // B)
        hb = c % (N_CORES // B)
        heads = range(hb * HPC, (hb + 1) * HPC)
        w_rows, b_rows = [], []
        for sect, scale in ((0, SCALE), (1, 1.0)):
            for h in heads:
                r0 = sect * DIM + h * DH
                w_rows.append(qkv_w[r0:r0 + DH] * scale)
                b_rows.append(qkv_b[r0:r0 + DH] * scale)
        wv_rows = [qkv_w[2 * DIM + h * DH:2 * DIM + (h + 1) * DH]
                   for h in heads]
        wall_c = np.ascontiguousarray(
            np.concatenate(w_rows + wv_rows, 0).T).astype(bf)
        bqk_c = np.ascontiguousarray(
            np.concatenate(b_rows, 0).reshape(4, 128).T).astype(np.float32)
        wo_c = np.ascontiguousarray(
            out_w[:, hb * HPC * DH:(hb + 1) * HPC * DH].T).astype(bf)
        in_maps.append({
            "xT": xT_b[b],
            "wall": wall_c,
            "bqk": bqk_c,
            "wo": wo_c,
            "maskT": maskT_np,
        })
    return in_maps, pi


def kernel(x, routes, qkv_w, qkv_b, out_w, out_b):
    from concourse.bass_utils import run_bass_kernel_spmd

    x = np.ascontiguousarray(np.asarray(x, np.float32))
    routes = np.asarray(routes)
    qkv_w = np.asarray(qkv_w, np.float32)
    qkv_b = np.asarray(qkv_b, np.float32)
    out_w = np.asarray(out_w, np.float32)
    out_b = np.asarray(out_b, np.float32)

    pi, rank, kr, a, nch = _geometry(routes)
    in_maps, pi = _prep_inputs(x, routes, qkv_w, qkv_b, out_w)
    nc = _get_module(a, nch)
    res = run_bass_kernel_spmd(nc, in_maps, core_ids=list(range(N_CORES)))

    # V bias folded into the output bias: out += (b_v @ out_w.T + out_b)
    bias_eff = qkv_b[2 * DIM:3 * DIM] @ out_w.T + out_b

    out = np.empty((B, S, DIM), np.float32)
    for b in range(B):
        cores = [c for c in range(N_CORES) if c // (N_CORES // B) == b]
        outT = res.results[cores[0]]["outp"].astype(np.float32)
        for c in cores[1:]:
            outT = outT + res.results[c]["outp"].astype(np.float32)
        rows_sorted = outT.T                      # [S, DIM] in rank order
        tmp = np.empty_like(rows_sorted)
        tmp[pi] = rows_sorted
        out[b] = tmp + bias_eff[None, :]
    return out


# revision 40
# speedup vs baseline: 1.8733x; 1.1028x over previous
"""Trainium2 Bass kernel for CantorAttention.

Strategy
--------
The Cantor routes are a pure function of the (quantized) Cantor value of each
position: sorting positions by that value makes every query's 64-key route set
live inside a narrow (<=385-wide) window of the sorted order.  Sparse
attention therefore becomes dense *banded* attention after a host-side
permutation.

This version computes the banded scores directly in transposed [key, query]
layout, one 128-key chunk at a time, which removes every probability
transpose from the hot path:

  host:   pi = argsort(cantor_val), permute x rows, transpose, cast bf16;
          build a per-query-tile 0/1 transposed key mask.
  device: Phase A: qT/kT projections ([d, s], bf16) and V in [s, d] layout
          with a constant-ones column per head (bias of V folded into the
          host-side output bias).
          Phase C: per (head, query-tile): S^T chunks = kT_chunk^T @ qT_tile
          on PE, exp on ACT, 0/1 mask multiply on DVE (bf16 2x/4x mode), then
          PV with the exp tile as the stationary operand: out[q, 64+1] where
          the +1 column (ones in V) accumulates the softmax denominator.
          Normalisation is then a per-partition reciprocal+scale.
          Per supertile: PE-transpose of the normalised attention output into
          [d, s] layout, then the output projection (Phase D).
  host:   sum the 4 partial outT blocks per batch, transpose, un-permute,
          add the (output + folded V) bias.

Sharding: batch x head-block -> 8 cores (core c: b = c//4, heads 4*(c%4)..).
"""

import sys

sys.path.insert(0, "/opt/trn_rl_repo")

import numpy as np

B, S, DIM = 2, 2048, 1024
HEADS, DH = 16, 64
K_NEI = 64
N_CORES = 8
HPC = 4            # heads per core
QT = 128           # query tile (rows per tile)
NT = S // QT       # 16 query tiles
SUP = 4            # query tiles per supertile
NSUP = NT // SUP
MAXCH = 3          # chunk slots per tile in the mask layout

_CACHE = {}


def _cantor_val(seq_len, depth=8):
    pos = np.arange(seq_len, dtype=np.float64)
    x = pos / max(1, seq_len - 1)
    x = np.clip(x, 1e-6, 1.0 - 1e-6)
    val = np.zeros_like(x)
    factor = 0.5
    for _ in range(depth):
        xs = x * 3.0
        digit = np.floor(xs)
        x = xs - digit
        val = val + (digit == 2.0).astype(np.float64) * factor
        factor *= 0.5
    return np.clip(val, 0.0, 1.0)


def _geometry(routes):
    """Window geometry from the runtime routes array.

    Returns (pi, rank, kr, a, nch): permutation, ranks, routed key ranks in
    query-rank order, per-tile 128-aligned window start (in chunks) and
    per-tile chunk count (<= MAXCH).
    """
    val = _cantor_val(S)
    pi = np.argsort(val, kind="stable").astype(np.int64)
    rank = np.empty(S, np.int64)
    rank[pi] = np.arange(S)
    kr = rank[np.asarray(routes, np.int64)][pi]      # [S, K] key ranks
    a = np.zeros(NT, np.int64)
    nch = np.zeros(NT, np.int64)
    for t in range(NT):
        l = int(kr[t * QT:(t + 1) * QT].min())
        h = int(kr[t * QT:(t + 1) * QT].max()) + 1
        a[t] = l // 128
        nch[t] = -(-(h - a[t] * 128) // 128)
        if nch[t] > MAXCH:
            raise ValueError("routes structure incompatible with banded kernel")
    return pi, rank, kr, a, nch


def _build_module(a, nch, loop_n=1):
    from contextlib import nullcontext

    from concourse import bacc, tile, mybir
    from concourse.masks import make_identity

    f32 = mybir.dt.float32
    bf16 = mybir.dt.bfloat16
    AF = mybir.ActivationFunctionType
    a = [int(v) for v in a]
    nch = [int(v) for v in nch]

    nc = bacc.Bacc("TRN2", target_bir_lowering=False, debug=False)
    xT = nc.dram_tensor("xT", [DIM, S], bf16, kind="ExternalInput").ap()
    wall = nc.dram_tensor("wall", [DIM, 3 * HPC * DH], bf16,
                          kind="ExternalInput").ap()
    bqk = nc.dram_tensor("bqk", [128, 4], f32, kind="ExternalInput").ap()
    wo = nc.dram_tensor("wo", [HPC * DH, DIM], bf16, kind="ExternalInput").ap()
    maskT = nc.dram_tensor("maskT", [QT, NT * MAXCH * QT], bf16,
                           kind="ExternalInput").ap()
    outp = nc.dram_tensor("outp", [DIM, S], bf16, kind="ExternalOutput").ap()

    NQK = 2 * HPC * DH                   # 512 rows of q|k in qkT
    NMT = NQK // 128                     # 4 row-tiles of qkT

    with tile.TileContext(nc) as tc:
        with tc.tile_pool(name="persist", bufs=1) as pp:
            id32 = pp.tile([128, 128], f32)
            make_identity(nc, id32)
            id_b = pp.tile([128, 128], bf16)
            nc.vector.tensor_copy(id_b, id32)
            maskT_sb = pp.tile([QT, NT * MAXCH * QT], bf16)
            bqk_sb = pp.tile([128, NMT], f32, tag="bqk", name="bqk")
            wo_sb = []
            for p2 in range(2):
                wt = pp.tile([128, DIM], bf16, tag=f"wo{p2}", name=f"wo{p2}")
                wo_sb.append(wt)
            # qT/kT: row-tile m holds heads 2m, 2m+1 (m 0-1: q, 2-3: k)
            qkT = [pp.tile([128, S], bf16, tag=f"qkT{m}", name=f"qkT{m}")
                   for m in range(NMT)]
            # V chunks in [s, d] layout: 4 heads x (64 V cols + ones col)
            V_sb = [pp.tile([128, HPC * (DH + 1)], bf16, tag=f"V{cc}",
                            name=f"V{cc}") for cc in range(NT)]
            for cc in range(NT):
                nc.gpsimd.memset(V_sb[cc], 1.0)
            attn_sb = pp.tile([QT, NT * HPC * DH], bf16, tag="attn",
                              name="attn")
            attn_outT = [pp.tile([128, S], bf16, tag=f"aout{p}",
                                 name=f"aout{p}") for p in range(2)]

            loop_cm = tc.For_i(0, loop_n, 1) if loop_n > 1 else nullcontext()
            with loop_cm:
                # one software-pipelined scope: A(u+1) overlaps C(u)/D(u)
                with tc.tile_pool(name="phA", bufs=1) as pa, \
                     tc.tile_pool(name="pexp", bufs=10) as pe_pool, \
                     tc.tile_pool(name="prec", bufs=4) as prec_pool, \
                     tc.tile_pool(name="pstg", bufs=2) as pstg_pool, \
                     tc.tile_pool(name="psA", bufs=2, space="PSUM") as psa, \
                     tc.tile_pool(name="psO", bufs=2, space="PSUM") as pso, \
                     tc.tile_pool(name="psD", bufs=4, space="PSUM") as psd:
                    wall_sb = []     # [wqk | wv] fused weight tiles
                    xt_sb = []
                    for kk in range(8):
                        wt = pa.tile([128, NQK + HPC * DH], bf16,
                                     tag=f"wall{kk}", name=f"wall{kk}")
                        nc.sync.dma_start(out=wt,
                                          in_=wall[kk * 128:(kk + 1) * 128, :])
                        wall_sb.append(wt)
                        t_ = pa.tile([128, S], bf16, tag=f"x{kk}",
                                     name=f"x{kk}")
                        nc.sync.dma_start(out=t_[:, 0:1024],
                                          in_=xT[kk * 128:(kk + 1) * 128,
                                                 0:1024])
                        xt_sb.append(t_)
                    nc.sync.dma_start(out=bqk_sb, in_=bqk)
                    for kk in range(8):
                        nc.sync.dma_start(out=xt_sb[kk][:, 1024:S],
                                          in_=xT[kk * 128:(kk + 1) * 128,
                                                 1024:S])
                    for p2 in range(2):
                        nc.sync.dma_start(out=wo_sb[p2],
                                          in_=wo[p2 * 128:(p2 + 1) * 128, :])
                    nc.sync.dma_start(out=maskT_sb, in_=maskT)

                    def phase_a(n):
                        for m in range(NMT):
                            ps = psa.tile([128, 512], f32, tag="ps",
                                          name=f"psa{n}_{m}")
                            for kk in range(8):
                                nc.tensor.matmul(
                                    ps, wall_sb[kk][:, m * 128:(m + 1) * 128],
                                    xt_sb[kk][:, n * 512:(n + 1) * 512],
                                    start=(kk == 0), stop=(kk == 7))
                            nc.vector.tensor_scalar_add(
                                qkT[m][:, n * 512:(n + 1) * 512], ps,
                                bqk_sb[m])
                        for j in range(4):
                            cc = n * 4 + j
                            pvt = psa.tile([128, 512], f32, tag="ps",
                                           name=f"psv{cc}")
                            pv = pvt[:, 0:HPC * DH]
                            for kk in range(8):
                                nc.tensor.matmul(
                                    pv, xt_sb[kk][:, cc * 128:(cc + 1) * 128],
                                    wall_sb[kk][:, NQK:], start=(kk == 0),
                                    stop=(kk == 7))
                            # scatter 4x64 head blocks into 4x65 slots
                            nc.gpsimd.tensor_copy(
                                V_sb[cc].rearrange("p (h e) -> p h e",
                                                   h=HPC)[:, :, 0:DH],
                                pv.rearrange("p (h e) -> p h e", h=HPC))

                    def a_qk(n, m):
                        ps = psa.tile([128, 512], f32, tag="ps",
                                      name=f"psa{n}_{m}")
                        for kk in range(8):
                            nc.tensor.matmul(
                                ps, wall_sb[kk][:, m * 128:(m + 1) * 128],
                                xt_sb[kk][:, n * 512:(n + 1) * 512],
                                start=(kk == 0), stop=(kk == 7))
                        nc.vector.tensor_scalar_add(
                            qkT[m][:, n * 512:(n + 1) * 512], ps,
                            bqk_sb[:, m:m + 1])

                    def a_v(cc):
                        pvt = psa.tile([128, 512], f32, tag="ps",
                                       name=f"psv{cc}")
                        pv = pvt[:, 0:HPC * DH]
                        for kk in range(8):
                            nc.tensor.matmul(
                                pv, xt_sb[kk][:, cc * 128:(cc + 1) * 128],
                                wall_sb[kk][:, NQK:], start=(kk == 0),
                                stop=(kk == 7))
                        # scatter 4x64 head blocks into 4x65 slots
                        nc.vector.tensor_copy(
                            V_sb[cc].rearrange("p (h e) -> p h e",
                                               h=HPC)[:, :, 0:DH],
                            pv.rearrange("p (h e) -> p h e", h=HPC))

                    def stage1(u, h, filler):
                        """S^T chunks + exp + mask for unit (u, h)."""
                        poff = (h % 2) * 64
                        qTh = qkT[h // 2]
                        kTh = qkT[2 + h // 2]
                        pexp = pe_pool.tile([128, SUP * MAXCH * 128], bf16,
                                            tag="pexp", name=f"pexp{u}_{h}")
                        for i, t in enumerate(range(u * SUP, (u + 1) * SUP)):
                            if i >= 2:
                                filler(i - 2)
                            st = psd.tile([128, MAXCH * 128], f32, tag="pod",
                                          name=f"st{h}_{t}")
                            for j in range(nch[t]):
                                c = a[t] + j
                                nc.tensor.matmul(
                                    st[:, j * 128:(j + 1) * 128],
                                    kTh[poff:poff + 64, c * 128:(c + 1) * 128],
                                    qTh[poff:poff + 64, t * 128:(t + 1) * 128],
                                    start=(j == 0), stop=(j == nch[t] - 1),
                                    skip_group_check=True)
                            w = nch[t] * 128
                            nc.scalar.activation(
                                out=pexp[:, i * MAXCH * 128:
                                         i * MAXCH * 128 + w],
                                in_=st[:, 0:w], func=AF.Exp)
                        # one masking multiply for the whole supertile row
                        nc.vector.tensor_mul(
                            pexp, pexp,
                            maskT_sb[:, u * SUP * MAXCH * 128:
                                     (u + 1) * SUP * MAXCH * 128])
                        return pexp

                    def stage2(u, h, pexp):
                        """PV + denominators + normalisation for unit (u, h)."""
                        po = pso.tile([128, SUP * (DH + 1)], f32, tag="po",
                                      name=f"po{h}_{u}")
                        first = True
                        for i, t in enumerate(range(u * SUP, (u + 1) * SUP)):
                            for j in range(nch[t]):
                                c = a[t] + j
                                last = (i == SUP - 1 and j == nch[t] - 1)
                                nc.tensor.matmul(
                                    po[:, i * (DH + 1):(i + 1) * (DH + 1)],
                                    pexp[:, (i * MAXCH + j) * 128:
                                         (i * MAXCH + j + 1) * 128],
                                    V_sb[c][:, h * (DH + 1):(h + 1) * (DH + 1)],
                                    start=first, stop=last,
                                    skip_group_check=True)
                                first = False
                        po3 = po.rearrange("p (i e) -> p i e", i=SUP)
                        rec = prec_pool.tile([128, SUP], f32, tag="rec",
                                             name=f"rec{h}_{u}")
                        nc.vector.reciprocal(rec, po3[:, :, DH:DH + 1])
                        att3 = attn_sb.rearrange("p (t e) -> p t e",
                                                 e=HPC * DH)
                        nc.vector.scalar_tensor_tensor(
                            att3[:, u * SUP:(u + 1) * SUP,
                                 h * DH:(h + 1) * DH],
                            po3[:, :, 0:DH], 1.0,
                            rec[:, :, None].to_broadcast((128, SUP, DH)),
                            op0=mybir.AluOpType.mult,
                            op1=mybir.AluOpType.mult)

                    def tr(u):
                        """Transpose attn [q, d] -> [d, q], 2 tiles per pass."""
                        for pair in range(2):
                            t0 = u * SUP + pair * 2
                            ptr = psd.tile([128, 512], bf16, tag="pod",
                                           name=f"ptr{t0}")
                            for z in range(2):
                                for p2 in range(2):
                                    nc.tensor.transpose(
                                        ptr[:, (z * 2 + p2) * 128:
                                            (z * 2 + p2 + 1) * 128],
                                        attn_sb[:, (t0 + z) * 256 + p2 * 128:
                                                (t0 + z) * 256 +
                                                (p2 + 1) * 128],
                                        id_b)
                            ptr3 = ptr.rearrange("p (z two q) -> p z two q",
                                                 z=2, two=2)
                            for p2 in range(2):
                                nc.vector.tensor_copy(
                                    attn_outT[p2][:, t0 * 128:(t0 + 2) * 128],
                                    ptr3[:, :, p2, :])

                    stg_tiles = {}

                    def d_mm(u, mm):
                        ps = psd.tile([128, 512], f32, tag="pod",
                                      name=f"psd{mm}_{u}")
                        for p2 in range(2):
                            nc.tensor.matmul(
                                ps, wo_sb[p2][:, mm * 128:(mm + 1) * 128],
                                attn_outT[p2][:, u * 512:(u + 1) * 512],
                                start=(p2 == 0), stop=(p2 == 1))
                        if u not in stg_tiles:
                            stg_tiles[u] = pstg_pool.tile(
                                [128, 8, 512], bf16, tag="stg", name=f"stg{u}")
                        stg = stg_tiles[u]
                        if mm % 2 == 0:
                            nc.scalar.copy(stg[:, mm, :], ps)
                        else:
                            nc.vector.tensor_copy(stg[:, mm, :], ps)
                        last = u == NSUP - 1
                        if mm == 7 or (last and mm % 2 == 1):
                            m0 = 2 * (mm // 2) if last else 0
                            nc.sync.dma_start(
                                out=outp.rearrange(
                                    "(mm p) c -> p mm c",
                                    p=128)[:, m0:mm + 1,
                                           u * 512:(u + 1) * 512],
                                in_=stg[:, m0:mm + 1, :])

                    # ---- deadline-scheduled software-pipelined emission ----
                    # Every A piece (qk row-tile or V chunk GEMM) gets the
                    # earliest C-slot that depends on it; pieces are emitted
                    # just before that slot, or earlier when PE has idle
                    # room.  D pieces fill remaining slack.
                    DELAY = 2
                    piece_fns = {}
                    deadline = {}
                    for n in range(4):
                        for m in range(NMT):
                            piece_fns[("m", n, m)] = (
                                lambda n=n, m=m: a_qk(n, m))
                        for j in range(4):
                            piece_fns[("v", n * 4 + j)] = (
                                lambda cc=n * 4 + j: a_v(cc))
                        deadline[("m", n, 0)] = 4 * n
                        deadline[("m", n, 1)] = 4 * n + 2
                        deadline[("m", n, 2)] = 99
                        deadline[("m", n, 3)] = 99
                    for u in range(NSUP):
                        tiles_u = range(u * SUP, (u + 1) * SUP)
                        cmin = min(a[t] for t in tiles_u)
                        cmax = max(a[t] + nch[t] - 1 for t in tiles_u)
                        for n in range(cmin // 4,
                                       ((cmax + 1) * 128 - 1) // 512 + 1):
                            deadline[("m", n, 2)] = min(
                                deadline[("m", n, 2)], 4 * u)
                            deadline[("m", n, 3)] = min(
                                deadline[("m", n, 3)], 4 * u + 2)
                        for cc in range(cmin, cmax + 1):
                            deadline[("v", cc)] = min(
                                deadline.get(("v", cc), 99), 4 * u + DELAY)
                    for cc in range(NT):
                        deadline.setdefault(("v", cc), 99)
                    queue = sorted(piece_fns,
                                   key=lambda k: (deadline[k], str(k)))
                    emitted = set()
                    fillers = []        # D pieces, no deadline

                    def emit_piece(key):
                        if key not in emitted:
                            emitted.add(key)
                            piece_fns[key]()

                    def drain(nmax):
                        done = 0
                        while done < nmax:
                            if queue:
                                emit_piece(queue.pop(0))
                            elif fillers:
                                fillers.pop(0)()
                            else:
                                break
                            done += 1

                    pending = []
                    for u in range(NSUP):
                        for h in range(HPC):
                            slot = u * HPC + h
                            while queue and deadline[queue[0]] <= slot:
                                emit_piece(queue.pop(0))
                            pexp = stage1(u, h, lambda i: drain(1))
                            pending.append((u, h, pexp))
                            if len(pending) > DELAY:
                                pu, ph, pe_ = pending.pop(0)
                                stage2(pu, ph, pe_)
                                if ph == HPC - 1:
                                    tr(pu)
                                    fillers.extend(
                                        lambda uu=pu, mm=mm: d_mm(uu, mm)
                                        for mm in range(8))
                            drain(1)
                    for pu, ph, pe_ in pending:
                        stage2(pu, ph, pe_)
                        if ph == HPC - 1:
                            tr(pu)
                            fillers.extend(
                                lambda uu=pu, mm=mm: d_mm(uu, mm)
                                for mm in range(8))
                    for key in list(queue):
                        emit_piece(key)
                    for f in fillers:
                        f()

    nc.compile()
    return nc


def _get_module(a, nch):
    key = (tuple(int(v) for v in a), tuple(int(v) for v in nch))
    if key not in _CACHE:
        _CACHE[key] = _build_module(a, nch)
    return _CACHE[key]


def _prep_inputs(x, routes, qkv_w, qkv_b, out_w):
    """Returns (in_maps, pi) for the 8 cores."""
    import ml_dtypes

    bf = ml_dtypes.bfloat16
    pi, rank, kr, a, nch = _geometry(routes)
    SCALE = 1.0 / float(np.sqrt(DH))

    # transposed 0/1 key mask: [kk partition, (t, j, qq) free]
    mask_np = np.zeros((NT, MAXCH, 128, QT), np.float32)
    t_all = np.arange(S) // QT
    col = kr - (a[t_all] * 128)[:, None]             # [S, K] window col
    rows = np.repeat(t_all, K_NEI)
    qq = np.repeat(np.arange(S) % QT, K_NEI)
    colr = col.ravel()
    mask_np[rows, colr // 128, colr % 128, qq] = 1.0
    maskT_np = np.ascontiguousarray(
        mask_np.transpose(2, 0, 1, 3).reshape(128, NT * MAXCH * 128)).astype(bf)

    xT_b = [np.ascontiguousarray(x[b][pi].T).astype(bf) for b in range(B)]

    in_maps = []
    for c in range(N_CORES):
        b = c // (N_CORES // B)
        hb = c % (N_CORES // B)
        heads = range(hb * HPC, (hb + 1) * HPC)
        w_rows, b_rows = [], []
        for sect, scale in ((0, SCALE), (1, 1.0)):
            for h in heads:
                r0 = sect * DIM + h * DH
                w_rows.append(qkv_w[r0:r0 + DH] * scale)
                b_rows.append(qkv_b[r0:r0 + DH] * scale)
        wv_rows = [qkv_w[2 * DIM + h * DH:2 * DIM + (h + 1) * DH]
                   for h in heads]
        wall_c = np.ascontiguousarray(
            np.concatenate(w_rows + wv_rows, 0).T).astype(bf)
        bqk_c = np.ascontiguousarray(
            np.concatenate(b_rows, 0).reshape(4, 128).T).astype(np.float32)
        wo_c = np.ascontiguousarray(
            out_w[:, hb * HPC * DH:(hb + 1) * HPC * DH].T).astype(bf)
        in_maps.append({
            "xT": xT_b[b],
            "wall": wall_c,
            "bqk": bqk_c,
            "wo": wo_c,
            "maskT": maskT_np,
        })
    return in_maps, pi


def kernel(x, routes, qkv_w, qkv_b, out_w, out_b):
    from concourse.bass_utils import run_bass_kernel_spmd

    x = np.ascontiguousarray(np.asarray(x, np.float32))
    routes = np.asarray(routes)
    qkv_w = np.asarray(qkv_w, np.float32)
    qkv_b = np.asarray(qkv_b, np.float32)
    out_w = np.asarray(out_w, np.float32)
    out_b = np.asarray(out_b, np.float32)

    pi, rank, kr, a, nch = _geometry(routes)
    in_maps, pi = _prep_inputs(x, routes, qkv_w, qkv_b, out_w)
    nc = _get_module(a, nch)
    res = run_bass_kernel_spmd(nc, in_maps, core_ids=list(range(N_CORES)))

    # V bias folded into the output bias: out += (b_v @ out_w.T + out_b)
    bias_eff = qkv_b[2 * DIM:3 * DIM] @ out_w.T + out_b

    out = np.empty((B, S, DIM), np.float32)
    for b in range(B):
        cores = [c for c in range(N_CORES) if c // (N_CORES // B) == b]
        outT = res.results[cores[0]]["outp"].astype(np.float32)
        for c in cores[1:]:
            outT = outT + res.results[c]["outp"].astype(np.float32)
        rows_sorted = outT.T                      # [S, DIM] in rank order
        tmp = np.empty_like(rows_sorted)
        tmp[pi] = rows_sorted
        out[b] = tmp + bias_eff[None, :]
    return out
